# revision 1
# baseline (speedup 1.0000x reference)
"""GCN2 (GCNII) message-passing kernel for 8 Trainium2 NeuronCores.

Strategy (1D node partition, hint-compliant):
  - Nodes sharded 8 ways by id; each core owns 12500 nodes (padded to 12544).
  - Layer weights replicated; per-layer halo exchange realized as 4 chunked
    AllGathers of each core's h' shard into a replicated DRAM node table.
  - Symmetric normalization folded into the data path:
      table rows store h' = dinv * h, per-edge coefficient v = 0.9*dinv[dst].
  - Aggregation (segment_sum over dst-sorted edges): dma_gather of 512B rows
    by src, then per-128-edge-chunk one-hot "indicator" matmuls on the
    TensorEngine accumulating into PSUM (S^T[feat,dst] += G^T @ I).
    Indicators built on the VectorEngine: (iota == dstlocal) * v, one fused
    tensor_scalar per chunk.
  - Edges grouped by (dst-tile of 128, src-bank of 32768 rows) because
    dma_gather indices are int16; chunk counts padded to the max across cores
    so one SPMD program serves all 8 cores.
"""

import math
import os
import sys

import numpy as np

for _p in ("/opt/trn_rl_repo",):
    if _p not in sys.path and os.path.isdir(_p):
        sys.path.insert(0, _p)

import concourse.bacc as bacc
import concourse.mybir as mybir
import concourse.tile as tile
from concourse.bass_utils import run_bass_kernel_spmd

# ---------------- problem constants (hardcoded per contract) ----------------
N = 100_000
E = 1_600_000
IN_C = 500
HID = 128
OUT_C = 64
L = 8
ALPHA = 0.1
THETA = 0.5

NCORES = 8
NOWN = N // NCORES          # 12500 real nodes per core
NLOC = 12544                # padded to 98 * 128
NT = NLOC // 128            # 98 dst tiles per core
TGS = 4                     # dst tiles per gather group
KIN = 512                   # padded input channels
BANK = 32768                # int16-addressable rows per gather bank
AGC = 4096                  # shard rows per chunked AllGather (AG 0..2)
TROWS = 3 * BANK + NCORES * (NLOC - 3 * AGC)   # 100352 table rows

F32 = mybir.dt.float32
I16 = mybir.dt.int16

_cache = {}

LAST_PERF = {}


def _row_of_node(n):
    """Table row of global node id(s) n (vectorized)."""
    c = n // NOWN
    i = n - c * NOWN
    q = np.minimum(i // AGC, 3)
    tail = NLOC - 3 * AGC  # 256
    return np.where(q < 3, q * BANK + c * AGC + (i - q * AGC),
                    3 * BANK + c * tail + (i - 3 * AGC))


def _preprocess(edge_index):
    """All graph-structure preprocessing on host (numpy)."""
    e0 = edge_index[0].astype(np.int64)
    e1 = edge_index[1].astype(np.int64)
    loop = np.arange(N, dtype=np.int64)
    src = np.concatenate([e0, loop])
    dst = np.concatenate([e1, loop])

    deg = np.bincount(dst, minlength=N).astype(np.float64)
    dinv = np.where(deg > 0, 1.0 / np.sqrt(deg), 0.0).astype(np.float32)

    owner = dst // NOWN
    ldst = dst - owner * NOWN
    t_arr = ldst // 128
    dl_arr = (ldst % 128).astype(np.float32)
    row = _row_of_node(src)
    b_arr = row // BANK
    bidx = (row % BANK).astype(np.int16)
    v_arr = ((1.0 - ALPHA) * dinv[dst]).astype(np.float32)

    # group id and stable sort
    G = ((owner * NT + t_arr) * 4 + b_arr).astype(np.int64)
    order = np.argsort(G, kind="stable")
    Gs = G[order]
    counts = np.bincount(Gs, minlength=NCORES * NT * 4).reshape(NCORES, NT, 4)
    C = np.ceil(counts / 128).astype(np.int64).max(axis=0)  # [NT, 4]

    # chunk stream plan (identical for all cores)
    tg_tiles = [list(range(g, min(g + TGS, NT))) for g in range(0, NT, TGS)]
    tg_meta = []
    slot0 = np.zeros((NT, 4), np.int64)
    ch = 0
    for tiles in tg_tiles:
        tg_ch0 = ch
        banks = []
        for b in range(4):
            b_off = ch - tg_ch0
            ents = []
            for t in tiles:
                slot0[t, b] = ch * 128
                if C[t, b] > 0:
                    ents.append((t, ch - tg_ch0, int(C[t, b])))
                ch += C[t, b]
            banks.append((int(b_off), int(ch - tg_ch0 - b_off), ents))
        tg_meta.append(dict(ch0=int(tg_ch0), nch=int(ch - tg_ch0),
                            banks=banks, tiles=tiles))
    NCHUNK = int(ch)
    NSLOT = NCHUNK * 128

    # fill per-core flat arrays (vectorized scatter)
    gstart = np.zeros(NCORES * NT * 4 + 1, np.int64)
    np.cumsum(counts.reshape(-1), out=gstart[1:])
    rank = np.arange(len(Gs)) - gstart[Gs]
    core_of = Gs // (NT * 4)
    tb = Gs % (NT * 4)
    dest = core_of * NSLOT + slot0.reshape(-1)[tb] + rank

    idx_flat = np.zeros(NCORES * NSLOT, np.int16)
    dl_flat = np.zeros(NCORES * NSLOT, np.float32)
    v_flat = np.zeros(NCORES * NSLOT, np.float32)
    idx_flat[dest] = bidx[order]
    dl_flat[dest] = dl_arr[order]
    v_flat[dest] = v_arr[order]

    idx_flat = idx_flat.reshape(NCORES, NSLOT)
    dl_flat = dl_flat.reshape(NCORES, NSLOT)
    v_flat = v_flat.reshape(NCORES, NSLOT)

    # device layouts
    idx_w = np.tile(
        idx_flat.reshape(NCORES, NSLOT // 16, 16).transpose(0, 2, 1), (1, 8, 1)
    ).copy()                                            # [c, 128, NSLOT/16]
    dl_w = dl_flat.reshape(NCORES, NCHUNK, 128).transpose(0, 2, 1).copy()
    v_w = v_flat.reshape(NCORES, NCHUNK, 128).transpose(0, 2, 1).copy()

    dinv_loc = np.zeros((NCORES, NLOC), np.float32)
    dinv_loc[:, :NOWN] = dinv.reshape(NCORES, NOWN)
    dinv_col = dinv_loc.reshape(NCORES, NT, 128).transpose(0, 2, 1).copy()

    return dict(tg_meta=tg_meta, NCHUNK=NCHUNK, NSLOT=NSLOT,
                idx_w=idx_w, dl_w=dl_w, v_w=v_w, dinv_col=dinv_col,
                counts=counts, C=C)


def _build_program(pre, n_layers=L, max_tg=None):
    nc = bacc.Bacc("TRN2", target_bir_lowering=False, debug=False,
                   num_devices=NCORES, num_swdge_queues=4)
    tg_meta = pre["tg_meta"]
    if max_tg is not None:
        tg_meta = tg_meta[:max_tg]
    gq = [0]  # round-robin SWDGE queue for gathers
    NCHUNK, NSLOT = pre["NCHUNK"], pre["NSLOT"]
    betas = [float(np.log(THETA / (l + 1) + 1.0)) for l in range(L)]

    # ---- external inputs ----
    xT_in = nc.dram_tensor("xT", [128, KIN // 128, NLOC], F32, kind="ExternalInput")
    win_in = nc.dram_tensor("win", [128, KIN // 128, HID], F32, kind="ExternalInput")
    bin_in = nc.dram_tensor("bin", [128, 1], F32, kind="ExternalInput")
    wc_in = nc.dram_tensor("wc", [128, L, HID], F32, kind="ExternalInput")
    wout_in = nc.dram_tensor("wout", [128, OUT_C], F32, kind="ExternalInput")
    bout_in = nc.dram_tensor("bout", [128, OUT_C], F32, kind="ExternalInput")
    iota_in = nc.dram_tensor("iota", [128, 128], F32, kind="ExternalInput")
    ident_in = nc.dram_tensor("ident", [128, 128], F32, kind="ExternalInput")
    dinv_in = nc.dram_tensor("dinvc", [128, NT], F32, kind="ExternalInput")
    idx_in = nc.dram_tensor("idx", [128, NSLOT // 16], I16, kind="ExternalInput")
    dl_in = nc.dram_tensor("dl", [128, NCHUNK], F32, kind="ExternalInput")
    v_in = nc.dram_tensor("v", [128, NCHUNK], F32, kind="ExternalInput")
    out_ext = nc.dram_tensor("out", [NOWN, OUT_C], F32, kind="ExternalOutput")

    rg = [list(range(NCORES))]

    with tile.TileContext(nc, num_cores=NCORES) as tc:
        with (
            tc.tile_pool(name="const", bufs=1) as cpool,
            tc.tile_pool(name="dram", bufs=1, space="DRAM") as dram,
            tc.tile_pool(name="work", bufs=1) as wp,
            tc.tile_pool(name="psum", bufs=1, space="PSUM") as pp,
        ):
            # ---- resident constants ----
            win_sb = cpool.tile([128, KIN // 128, HID], F32)
            bin_sb = cpool.tile([128, 1], F32)
            wc_sb = cpool.tile([128, L, HID], F32)
            wout_sb = cpool.tile([128, OUT_C], F32)
            bout_sb = cpool.tile([128, OUT_C], F32)
            iota_sb = cpool.tile([128, 128], F32)
            ident_sb = cpool.tile([128, 128], F32)
            dinv_sb = cpool.tile([128, NT], F32)
            h0sT = cpool.tile([128, NLOC], F32)   # 0.1 * h0^T resident
            for sb_t, ext in ((win_sb, win_in), (bin_sb, bin_in), (wc_sb, wc_in),
                              (wout_sb, wout_in), (bout_sb, bout_in),
                              (iota_sb, iota_in), (ident_sb, ident_in),
                              (dinv_sb, dinv_in)):
                nc.sync.dma_start(sb_t[:], ext[:])

            # one Shared table tensor per (allgather round, bank): a Shared
            # tensor may only have a single writer instruction.
            NBROWS = [BANK, BANK, BANK, TROWS - 3 * BANK]
            tables = [
                [dram.tile([NBROWS[b], HID], F32, addr_space="Shared",
                           name=f"table{r}_{b}") for b in range(4)]
                for r in range(L)
            ]
            shards = [
                dram.tile([NLOC, HID], F32, name=f"shard{i}") for i in range(2)
            ]

            def bank_view(tb, b):
                return tb[b][:, :]

            def ag(shard, tb):
                """Chunked AllGather shard -> table (4 calls)."""
                bounds = [(0, AGC), (AGC, 2 * AGC), (2 * AGC, 3 * AGC),
                          (3 * AGC, NLOC)]
                for b, (r0, r1) in enumerate(bounds):
                    nc.gpsimd.collective_compute(
                        "AllGather", mybir.AluOpType.bypass, replica_groups=rg,
                        ins=[shard[r0:r1, :].opt()],
                        outs=[bank_view(tb, b).opt()],
                    )

            def write_rows(hT_sb, width, t):
                """transpose + dinv-scale + store rows of tile t to shard."""
                pst = pp.tile([128, 128], F32, tag="pst", bufs=2)
                nc.tensor.transpose(pst[:width, :], hT_sb[:, :width], ident_sb[:])
                rows = wp.tile([128, HID], F32, tag="rows", bufs=3)
                nc.vector.tensor_scalar(
                    rows[:width, :], pst[:width, :], dinv_sb[:width, t:t + 1],
                    None, mybir.AluOpType.mult)
                return rows

            # ================= input layer: h0 = relu(x @ W_in + b_in) ======
            shard0 = shards[0]
            for g0 in range(0, NLOC, 512):
                w = min(512, NLOC - g0)
                psin = pp.tile([128, 512], F32, tag="psz", bufs=2)
                for k in range(KIN // 128):
                    xt = wp.tile([128, 512], F32, tag="xt", bufs=3)
                    nc.sync.dma_start(xt[:, :w], xT_in[:, k, g0:g0 + w])
                    nc.tensor.matmul(psin[:, :w], win_sb[:, k, :], xt[:, :w],
                                     start=(k == 0), stop=(k == KIN // 128 - 1))
                h0T = wp.tile([128, 512], F32, tag="h0t", bufs=2)
                nc.scalar.activation(h0T[:, :w], psin[:, :w],
                                     mybir.ActivationFunctionType.Relu,
                                     bias=bin_sb[:, 0:1])
                nc.vector.tensor_scalar(h0sT[:, g0:g0 + w], h0T[:, :w], ALPHA,
                                        None, mybir.AluOpType.mult)
                for j in range(0, w, 128):
                    t = (g0 + j) // 128
                    rows = write_rows(h0T[:, j:j + 128], 128, t)
                    nc.sync.dma_start(shard0[t * 128:(t + 1) * 128, :], rows[:])
            ag(shard0, tables[0])

            # ========================= L layers =============================
            for l in range(n_layers):
                t_in = tables[l]
                shard = shards[(l + 1) % 2]
                last = l == L - 1
                for tg in tg_meta:
                    nch = tg["nch"]
                    ch0 = tg["ch0"]
                    idx_t = wp.tile([128, nch * 8], I16, tag="idxs", bufs=3)
                    nc.sync.dma_start(idx_t[:], idx_in[:, ch0 * 8:(ch0 + nch) * 8])
                    dl_t = wp.tile([128, nch], F32, tag="dlt", bufs=3)
                    nc.sync.dma_start(dl_t[:], dl_in[:, ch0:ch0 + nch])
                    v_t = wp.tile([128, nch], F32, tag="vt", bufs=3)
                    nc.sync.dma_start(v_t[:], v_in[:, ch0:ch0 + nch])
                    skips = os.environ.get("GCN_SKIP", "").split(",")
                    gbuf = wp.tile([128, nch, HID], F32, tag="g", bufs=2)
                    per_tile = {t: [] for t in tg["tiles"]}
                    for b, (b_off, b_nch, ents) in enumerate(tg["banks"]):
                        if b_nch == 0:
                            continue
                        if "gather" not in skips:
                            nc.gpsimd.dma_gather(
                                gbuf[:, b_off:b_off + b_nch, :],
                                bank_view(t_in, b),
                                idx_t[:, b_off * 8:(b_off + b_nch) * 8],
                                b_nch * 128, b_nch * 128, HID,
                                single_packet=False,
                                queue_num=gq[0] % 4,
                            )
                            gq[0] += 1
                        for (t, pos, cnt) in ents:
                            per_tile[t].append((pos, cnt))

                    for t in tg["tiles"]:
                        chunks = [p for (pos, cnt) in per_tile[t]
                                  for p in range(pos, pos + cnt)]
                        if "mm" in skips:
                            chunks = []
                        ps = pp.tile([128, 128], F32, tag="ps", bufs=3)
                        for k, p in enumerate(chunks):
                            if "ind" not in skips:
                                ind = wp.tile([128, 128], F32, tag="ind", bufs=4)
                                nc.vector.tensor_scalar(
                                    ind[:], iota_sb[:], dl_t[:, p:p + 1],
                                    v_t[:, p:p + 1], mybir.AluOpType.is_equal,
                                    mybir.AluOpType.mult)
                                rhs = ind[:]
                            else:
                                rhs = iota_sb[:]
                            nc.tensor.matmul(ps[:], gbuf[:, p, :], rhs,
                                             start=(k == 0),
                                             stop=(k == len(chunks) - 1))
                        # z^T = S^T + 0.1*h0^T
                        z = wp.tile([128, 128], F32, tag="z", bufs=3)
                        if chunks:
                            nc.vector.tensor_tensor(
                                z[:], ps[:], h0sT[:, t * 128:(t + 1) * 128],
                                mybir.AluOpType.add)
                        else:
                            nc.vector.tensor_copy(
                                z[:], h0sT[:, t * 128:(t + 1) * 128])
                        if not last:
                            psz = pp.tile([128, 128], F32, tag="psz", bufs=2)
                            nc.tensor.matmul(psz[:], wc_sb[:, l, :], z[:],
                                             start=True, stop=True)
                            pre = wp.tile([128, 128], F32, tag="pre", bufs=2)
                            nc.vector.scalar_tensor_tensor(
                                pre[:], z[:], 1.0 - betas[l], psz[:],
                                mybir.AluOpType.mult, mybir.AluOpType.add)
                            hT = wp.tile([128, 128], F32, tag="ht", bufs=2)
                            nc.scalar.activation(
                                hT[:], pre[:], mybir.ActivationFunctionType.Relu)
                            rows = write_rows(hT, 128, t)
                            nc.sync.dma_start(
                                shard[t * 128:(t + 1) * 128, :], rows[:])
                        else:
                            psz = pp.tile([128, 128], F32, tag="psz", bufs=2)
                            nc.tensor.matmul(psz[:], wc_sb[:, l, :], z[:],
                                             start=True, stop=True)
                            pre = wp.tile([128, 128], F32, tag="pre", bufs=2)
                            nc.vector.scalar_tensor_tensor(
                                pre[:], z[:], 1.0 - betas[l], psz[:],
                                mybir.AluOpType.mult, mybir.AluOpType.add)
                            hT = wp.tile([128, 128], F32, tag="ht", bufs=2)
                            nc.scalar.activation(
                                hT[:], pre[:], mybir.ActivationFunctionType.Relu)
                            # output: out = h @ W_out + b_out
                            pso = pp.tile([64, 128], F32, tag="ps", bufs=3)
                            nc.tensor.matmul(pso[:], wout_sb[:], hT[:],
                                             start=True, stop=True)
                            oT = wp.tile([64, 128], F32, tag="ot", bufs=2)
                            nc.vector.tensor_copy(oT[:], pso[:])
                            psq = pp.tile([128, 64], F32, tag="pst", bufs=2)
                            nc.tensor.transpose(psq[:], oT[:],
                                                ident_sb[:64, :64])
                            ob = wp.tile([128, 64], F32, tag="ob", bufs=3)
                            nc.vector.tensor_tensor(ob[:], psq[:], bout_sb[:],
                                                    mybir.AluOpType.add)
                            r0 = t * 128
                            r1 = min(r0 + 128, NOWN)
                            if r1 > r0:
                                nc.sync.dma_start(out_ext[r0:r1, :],
                                                  ob[:r1 - r0, :])
                    # end tiles
                if not last:
                    ag(shard, tables[l + 1])

    nc.compile()
    return nc


def _host_inputs(inputs, pre):
    x = np.asarray(inputs["x"], np.float32)
    W_in = np.asarray(inputs["W_in"], np.float32)
    b_in = np.asarray(inputs["b_in"], np.float32)
    W_conv = np.asarray(inputs["W_conv"], np.float32)
    W_out = np.asarray(inputs["W_out"], np.float32)
    b_out = np.asarray(inputs["b_out"], np.float32)
    betas = np.array([math.log(THETA / (l + 1) + 1.0) for l in range(L)],
                     np.float32)

    win_w = np.zeros((128, KIN // 128, HID), np.float32)
    for k in range(KIN // 128):
        rows = W_in[k * 128:min((k + 1) * 128, IN_C)]
        win_w[:rows.shape[0], k, :] = rows
    wc_w = (W_conv * betas[:, None, None]).transpose(1, 0, 2).copy()  # [128,L,128]
    iota_w = np.tile(np.arange(128, dtype=np.float32)[None, :], (128, 1))
    ident_w = np.eye(128, dtype=np.float32)
    bout_w = np.tile(b_out[None, :], (128, 1)).astype(np.float32)
    bin_w = b_in.reshape(128, 1).astype(np.float32)

    xT_w = np.zeros((NCORES, 128, KIN // 128, NLOC), np.float32)
    xr = x.reshape(NCORES, NOWN, IN_C)
    for k in range(KIN // 128):
        c0, c1 = k * 128, min((k + 1) * 128, IN_C)
        xT_w[:, :c1 - c0, k, :NOWN] = xr[:, :, c0:c1].transpose(0, 2, 1)

    maps = []
    for c in range(NCORES):
        maps.append({
            "xT": xT_w[c], "win": win_w, "bin": bin_w, "wc": wc_w,
            "wout": W_out, "bout": bout_w, "iota": iota_w, "ident": ident_w,
            "dinvc": pre["dinv_col"][c], "idx": pre["idx_w"][c],
            "dl": pre["dl_w"][c], "v": pre["v_w"][c],
        })
    return maps


def kernel(**inputs):
    edge_index = np.asarray(inputs["edge_index"])
    key = hash(edge_index.tobytes())
    if key not in _cache:
        pre = _preprocess(edge_index)
        n_layers = int(os.environ.get("GCN_NL", L))
        max_tg = os.environ.get("GCN_MAXTG")
        nc = _build_program(pre, n_layers,
                            int(max_tg) if max_tg else None)
        _cache.clear()
        _cache[key] = (pre, nc)
    pre, nc = _cache[key]

    in_maps = _host_inputs(inputs, pre)
    trace = bool(os.environ.get("GCN_TRACE"))
    res = run_bass_kernel_spmd(nc, in_maps, core_ids=list(range(NCORES)),
                               trace=trace)
    LAST_PERF["exec_time_ns"] = res.exec_time_ns
    LAST_PERF["mean_exec_time_ns"] = res.mean_exec_time_ns
    LAST_PERF["trace"] = (res.instructions_and_trace or (None, None))[1]
    out = np.concatenate([res.results[c]["out"] for c in range(NCORES)], axis=0)
    return out.astype(np.float32)



# revision 4
# speedup vs baseline: 1.0543x; 1.0543x over previous
"""GCN2 (GCNII) message-passing kernel for 8 Trainium2 NeuronCores.

Strategy (1D node partition):
  - Nodes sharded 8 ways by id; each core owns 12500 nodes (padded to 12544).
  - Layer weights replicated; per-layer halo exchange as 4 chunked AllGathers
    of each core's h' shard into a replicated DRAM node table (bf16).
  - Table rows store h' = dinv * h in bf16 (256B rows) — halves gather and
    collective traffic vs fp32.
  - Aggregation (segment_sum over dst-sorted edges): dma_gather of 256B rows
    by src, then per-128-edge-chunk one-hot matmuls on the TensorEngine
    accumulating into PSUM (S^T[feat,dst] += G^T @ I01).
  - Indicators are PURE 0/1 and built in BULK: one vector tensor_tensor
    (is_equal) per tile-group over [128, nch*128] with broadcast APs
    (iota broadcast along chunks, dl broadcast along the 128 lane dim).
    Padded edge slots carry dl=200 which matches no lane. tensor_tensor
    never uses the DVE 2-port mode, so it does not take the SBUF port lock
    that serializes against GpSimd SWDGE descriptor generation.
  - The per-edge norm v = 0.9*dinv[dst] depends only on dst, so it commutes
    through the layer: with a_d = 0.9*dinv_d,
        h'_next_col_d = dinv_d*a_d * relu(U_d + T'_d)
    where U = (1-b)S + b W^T S  and  T' = ((1-b) + b W^T) h0a,
    h0a = 0.1*h0/a (resident bf16). The per-partition scales (0.9*dinv^2,
    1/(9*dinv)) are applied on the Scalar engine post-transpose.
  - Scalar engine (idle otherwise) does all PSUM->SBUF copies and relu.
"""

import math
import os
import sys

import numpy as np
import ml_dtypes

for _p in ("/opt/trn_rl_repo",):
    if _p not in sys.path and os.path.isdir(_p):
        sys.path.insert(0, _p)

import concourse.bacc as bacc
import concourse.mybir as mybir
import concourse.tile as tile
from concourse.bass_utils import run_bass_kernel_spmd

# ---------------- problem constants (hardcoded per contract) ----------------
N = 100_000
E = 1_600_000
IN_C = 500
HID = 128
OUT_C = 64
L = 8
ALPHA = 0.1
THETA = 0.5

NCORES = 8
NOWN = N // NCORES          # 12500 real nodes per core
NLOC = 12544                # padded to 98 * 128
NT = NLOC // 128            # 98 dst tiles per core
TGS = 4                     # dst tiles per gather group
KIN = 512                   # padded input channels
BANK = 32768                # int16-addressable rows per gather bank
AGC = 4096                  # shard rows per chunked AllGather (AG 0..2)
TROWS = 3 * BANK + NCORES * (NLOC - 3 * AGC)   # 100352 table rows
PAD_DL = 200.0              # dl sentinel for padded edge slots (matches no lane)

F32 = mybir.dt.float32
BF16 = mybir.dt.bfloat16
I16 = mybir.dt.int16
BF = ml_dtypes.bfloat16

_cache = {}

LAST_PERF = {}


def _row_of_node(n):
    """Table row of global node id(s) n (vectorized)."""
    c = n // NOWN
    i = n - c * NOWN
    q = np.minimum(i // AGC, 3)
    tail = NLOC - 3 * AGC  # 256
    return np.where(q < 3, q * BANK + c * AGC + (i - q * AGC),
                    3 * BANK + c * tail + (i - 3 * AGC))


def _preprocess(edge_index):
    """All graph-structure preprocessing on host (numpy)."""
    e0 = edge_index[0].astype(np.int64)
    e1 = edge_index[1].astype(np.int64)
    loop = np.arange(N, dtype=np.int64)
    src = np.concatenate([e0, loop])
    dst = np.concatenate([e1, loop])

    deg = np.bincount(dst, minlength=N).astype(np.float64)
    dinv = np.where(deg > 0, 1.0 / np.sqrt(deg), 0.0).astype(np.float32)

    owner = dst // NOWN
    ldst = dst - owner * NOWN
    t_arr = ldst // 128
    dl_arr = (ldst % 128).astype(np.float32)
    row = _row_of_node(src)
    b_arr = row // BANK
    bidx = (row % BANK).astype(np.int16)

    # group id and stable sort
    G = ((owner * NT + t_arr) * 4 + b_arr).astype(np.int64)
    order = np.argsort(G, kind="stable")
    Gs = G[order]
    counts = np.bincount(Gs, minlength=NCORES * NT * 4).reshape(NCORES, NT, 4)
    C = np.ceil(counts / 128).astype(np.int64).max(axis=0)  # [NT, 4]

    # chunk stream plan (identical for all cores)
    tg_tiles = [list(range(g, min(g + TGS, NT))) for g in range(0, NT, TGS)]
    tg_meta = []
    slot0 = np.zeros((NT, 4), np.int64)
    ch = 0
    for tiles in tg_tiles:
        tg_ch0 = ch
        banks = []
        for b in range(4):
            b_off = ch - tg_ch0
            ents = []
            for t in tiles:
                slot0[t, b] = ch * 128
                if C[t, b] > 0:
                    ents.append((t, ch - tg_ch0, int(C[t, b])))
                ch += C[t, b]
            banks.append((int(b_off), int(ch - tg_ch0 - b_off), ents))
        tg_meta.append(dict(ch0=int(tg_ch0), nch=int(ch - tg_ch0),
                            banks=banks, tiles=tiles))
    NCHUNK = int(ch)
    NSLOT = NCHUNK * 128

    # fill per-core flat arrays (vectorized scatter)
    gstart = np.zeros(NCORES * NT * 4 + 1, np.int64)
    np.cumsum(counts.reshape(-1), out=gstart[1:])
    rank = np.arange(len(Gs)) - gstart[Gs]
    core_of = Gs // (NT * 4)
    tb = Gs % (NT * 4)
    dest = core_of * NSLOT + slot0.reshape(-1)[tb] + rank

    idx_flat = np.zeros(NCORES * NSLOT, np.int16)
    dl_flat = np.full(NCORES * NSLOT, PAD_DL, np.float32)
    idx_flat[dest] = bidx[order]
    dl_flat[dest] = dl_arr[order]

    idx_flat = idx_flat.reshape(NCORES, NSLOT)
    dl_flat = dl_flat.reshape(NCORES, NSLOT)

    # device layouts
    idx_w = np.tile(
        idx_flat.reshape(NCORES, NSLOT // 16, 16).transpose(0, 2, 1), (1, 8, 1)
    ).copy()                                            # [c, 128, NSLOT/16]
    dl_w = dl_flat.reshape(NCORES, NCHUNK, 128).transpose(0, 2, 1)
    dl_w = dl_w.astype(BF).copy()                       # [c, 128, NCHUNK]

    dinv_loc = np.zeros((NCORES, NLOC), np.float32)
    dinv_loc[:, :NOWN] = dinv.reshape(NCORES, NOWN)
    dcol = dinv_loc.reshape(NCORES, NT, 128).transpose(0, 2, 1).copy()
    acol = (0.9 * dcol).astype(np.float32)
    scol = (0.9 * dcol * dcol).astype(np.float32)
    hacol = np.where(dcol > 0, 1.0 / (9.0 * np.maximum(dcol, 1e-30)),
                     0.0).astype(np.float32)

    return dict(tg_meta=tg_meta, NCHUNK=NCHUNK, NSLOT=NSLOT,
                idx_w=idx_w, dl_w=dl_w, dinv_col=dcol, acol=acol,
                scol=scol, hacol=hacol, counts=counts, C=C)


def _build_program(pre, n_layers=L, max_tg=None):
    nc = bacc.Bacc("TRN2", target_bir_lowering=False, debug=False,
                   num_devices=NCORES, num_swdge_queues=4)
    tg_meta = pre["tg_meta"]
    if max_tg is not None:
        tg_meta = tg_meta[:max_tg]
    gq = [0]  # round-robin SWDGE queue for gathers
    NCHUNK, NSLOT = pre["NCHUNK"], pre["NSLOT"]
    betas = [float(np.log(THETA / (l + 1) + 1.0)) for l in range(L)]
    skips = os.environ.get("GCN_SKIP", "").split(",")

    # ---- external inputs ----
    xT_in = nc.dram_tensor("xT", [128, KIN // 128, NLOC], BF16, kind="ExternalInput")
    win_in = nc.dram_tensor("win", [128, KIN // 128, HID], BF16, kind="ExternalInput")
    bin_in = nc.dram_tensor("bin", [128, 1], F32, kind="ExternalInput")
    wc_in = nc.dram_tensor("wc", [128, L, HID], BF16, kind="ExternalInput")
    wout_in = nc.dram_tensor("wout", [128, OUT_C], BF16, kind="ExternalInput")
    bout_in = nc.dram_tensor("bout", [128, OUT_C], BF16, kind="ExternalInput")
    iota_in = nc.dram_tensor("iota", [128, 128], BF16, kind="ExternalInput")
    ident_in = nc.dram_tensor("ident", [128, 128], BF16, kind="ExternalInput")
    dinv_in = nc.dram_tensor("dinvc", [128, NT], F32, kind="ExternalInput")
    acol_in = nc.dram_tensor("acol", [128, NT], F32, kind="ExternalInput")
    scol_in = nc.dram_tensor("scol", [128, NT], F32, kind="ExternalInput")
    hacol_in = nc.dram_tensor("hacol", [128, NT], F32, kind="ExternalInput")
    idx_in = nc.dram_tensor("idx", [128, NSLOT // 16], I16, kind="ExternalInput")
    dl_in = nc.dram_tensor("dl", [128, NCHUNK], BF16, kind="ExternalInput")
    out_ext = nc.dram_tensor("out", [NOWN, OUT_C], F32, kind="ExternalOutput")

    rg = [list(range(NCORES))]

    with tile.TileContext(nc, num_cores=NCORES) as tc:
        with (
            tc.tile_pool(name="const", bufs=1) as cpool,
            tc.tile_pool(name="dram", bufs=1, space="DRAM") as dram,
            tc.tile_pool(name="work", bufs=1) as wp,
            tc.tile_pool(name="psum", bufs=1, space="PSUM") as pp,
        ):
            # ---- resident constants ----
            win_sb = cpool.tile([128, KIN // 128, HID], BF16)
            bin_sb = cpool.tile([128, 1], F32)
            wc_sb = cpool.tile([128, L, HID], BF16)
            wout_sb = cpool.tile([128, OUT_C], BF16)
            bout_sb = cpool.tile([128, OUT_C], BF16)
            iota_sb = cpool.tile([128, 128], BF16)
            ident_sb = cpool.tile([128, 128], BF16)
            dinv_sb = cpool.tile([128, NT], F32)
            acol_sb = cpool.tile([128, NT], F32)
            scol_sb = cpool.tile([128, NT], F32)
            hacol_sb = cpool.tile([128, NT], F32)
            h0a = cpool.tile([128, NLOC], BF16)     # 0.1*h0/a resident
            tprime = cpool.tile([128, NLOC], BF16)  # per-layer T' buffer
            for sb_t, ext in ((win_sb, win_in), (bin_sb, bin_in), (wc_sb, wc_in),
                              (wout_sb, wout_in), (bout_sb, bout_in),
                              (iota_sb, iota_in), (ident_sb, ident_in),
                              (dinv_sb, dinv_in), (acol_sb, acol_in),
                              (scol_sb, scol_in), (hacol_sb, hacol_in)):
                nc.sync.dma_start(sb_t[:], ext[:])

            # one Shared table tensor per (allgather round, bank)
            NBROWS = [BANK, BANK, BANK, TROWS - 3 * BANK]
            tables = [
                [dram.tile([NBROWS[b], HID], BF16, addr_space="Shared",
                           name=f"table{r}_{b}") for b in range(4)]
                for r in range(L)
            ]
            shards = [
                dram.tile([NLOC, HID], BF16, name=f"shard{i}") for i in range(2)
            ]

            def ag(shard, tb):
                """Chunked AllGather shard -> table (4 calls)."""
                bounds = [(0, AGC), (AGC, 2 * AGC), (2 * AGC, 3 * AGC),
                          (3 * AGC, NLOC)]
                for b, (r0, r1) in enumerate(bounds):
                    nc.gpsimd.collective_compute(
                        "AllGather", mybir.AluOpType.bypass, replica_groups=rg,
                        ins=[shard[r0:r1, :].opt()],
                        outs=[tb[b][:, :].opt()],
                    )

            # ========= input layer: h0 = relu(x @ W_in + b_in) =========
            # Produces: shard0 rows (dinv*h0, bf16) and resident h0a
            # (= 0.1*h0/a, bf16, [feat, node]).
            shard0 = shards[0]
            for g0 in range(0, NLOC, 512):
                w = min(512, NLOC - g0)
                psin = pp.tile([128, 512], F32, tag="big", bufs=2)
                for k in range(KIN // 128):
                    xt = wp.tile([128, 512], BF16, tag="xt", bufs=3)
                    nc.sync.dma_start(xt[:, :w], xT_in[:, k, g0:g0 + w])
                    nc.tensor.matmul(psin[:, :w], win_sb[:, k, :], xt[:, :w],
                                     start=(k == 0), stop=(k == KIN // 128 - 1))
                h0T = wp.tile([128, 512], BF16, tag="h0t", bufs=2)
                nc.scalar.activation(h0T[:, :w], psin[:, :w],
                                     mybir.ActivationFunctionType.Relu,
                                     bias=bin_sb[:, 0:1])
                for j in range(0, w, 128):
                    t = (g0 + j) // 128
                    pst = pp.tile([128, 128], BF16, tag="pst", bufs=3)
                    nc.tensor.transpose(pst[:], h0T[:, j:j + 128], ident_sb[:])
                    rows0 = wp.tile([128, HID], BF16, tag="rows", bufs=3)
                    nc.scalar.activation(rows0[:], pst[:],
                                         mybir.ActivationFunctionType.Copy,
                                         scale=dinv_sb[:, t:t + 1])
                    nc.sync.dma_start(shard0[t * 128:(t + 1) * 128, :], rows0[:])
                    h0ar = wp.tile([128, HID], BF16, tag="h0ar", bufs=2)
                    nc.scalar.activation(h0ar[:], pst[:],
                                         mybir.ActivationFunctionType.Copy,
                                         scale=hacol_sb[:, t:t + 1])
                    ps2 = pp.tile([128, 128], BF16, tag="ps", bufs=3)
                    nc.tensor.transpose(ps2[:], h0ar[:], ident_sb[:])
                    nc.scalar.activation(h0a[:, t * 128:(t + 1) * 128], ps2[:],
                                         mybir.ActivationFunctionType.Copy)
            ag(shard0, tables[0])

            # ========================= L layers =============================
            for l in range(n_layers):
                t_in = tables[l]
                shard = shards[(l + 1) % 2]
                last = l == L - 1
                ob = 1.0 - betas[l]

                # T' = (1-b)*h0a + b*(W^T h0a)  (wc already holds b*W)
                for g0 in range(0, NLOC, 512):
                    w = min(512, NLOC - g0)
                    psT = pp.tile([128, 512], F32, tag="big", bufs=2)
                    nc.tensor.matmul(psT[:, :w], wc_sb[:, l, :],
                                     h0a[:, g0:g0 + w], start=True, stop=True)
                    tpb = wp.tile([128, 512], BF16, tag="tpb", bufs=2)
                    nc.scalar.activation(tpb[:, :w], psT[:, :w],
                                         mybir.ActivationFunctionType.Copy)
                    nc.vector.scalar_tensor_tensor(
                        tprime[:, g0:g0 + w], h0a[:, g0:g0 + w], ob,
                        tpb[:, :w], mybir.AluOpType.mult, mybir.AluOpType.add)

                for tg in tg_meta:
                    nch = tg["nch"]
                    ch0 = tg["ch0"]
                    idx_t = wp.tile([128, nch * 8], I16, tag="idxs", bufs=3)
                    nc.sync.dma_start(idx_t[:], idx_in[:, ch0 * 8:(ch0 + nch) * 8])
                    dl_t = wp.tile([128, nch], BF16, tag="dlt", bufs=3)
                    nc.sync.dma_start(dl_t[:], dl_in[:, ch0:ch0 + nch])

                    # bulk 0/1 indicator build: one DVE tensor_tensor per tg
                    ind = wp.tile([128, nch, 128], BF16, tag="ind", bufs=2)
                    nc.vector.tensor_tensor(
                        ind[:, :, :],
                        iota_sb[:, :].unsqueeze(1).broadcast_to([128, nch, 128]),
                        dl_t[:, :].unsqueeze(2).broadcast_to([128, nch, 128]),
                        mybir.AluOpType.is_equal)

                    gbuf = wp.tile([128, nch, HID], BF16, tag="g", bufs=2)
                    per_tile = {t: [] for t in tg["tiles"]}
                    for b, (b_off, b_nch, ents) in enumerate(tg["banks"]):
                        if b_nch == 0:
                            continue
                        if "gather" not in skips:
                            nc.gpsimd.dma_gather(
                                gbuf[:, b_off:b_off + b_nch, :],
                                t_in[b][:, :],
                                idx_t[:, b_off * 8:(b_off + b_nch) * 8],
                                b_nch * 128, b_nch * 128, HID,
                                single_packet=False,
                                queue_num=gq[0] % 4,
                            )
                            gq[0] += 1
                        for (t, pos, cnt) in ents:
                            per_tile[t].append((pos, cnt))

                    for t in tg["tiles"]:
                        chunks = [p for (pos, cnt) in per_tile[t]
                                  for p in range(pos, pos + cnt)]
                        if "mm" in skips:
                            chunks = []
                        psS = pp.tile([128, 128], F32, tag="ps", bufs=3)
                        for k, p in enumerate(chunks):
                            nc.tensor.matmul(psS[:], gbuf[:, p, :], ind[:, p, :],
                                             start=(k == 0),
                                             stop=(k == len(chunks) - 1))
                        # U path: psU = b*W^T S + T'
                        psU = pp.tile([128, 128], F32, tag="pst", bufs=3)
                        if chunks:
                            S_sb = wp.tile([128, 128], BF16, tag="ssb", bufs=3)
                            nc.scalar.activation(
                                S_sb[:], psS[:],
                                mybir.ActivationFunctionType.Copy)
                            nc.tensor.matmul(psU[:], wc_sb[:, l, :], S_sb[:],
                                             start=True, stop=False)
                            nc.tensor.matmul(psU[:], ident_sb[:],
                                             tprime[:, t * 128:(t + 1) * 128],
                                             start=False, stop=True)
                            Ub = wp.tile([128, 128], BF16, tag="ub", bufs=3)
                            nc.scalar.activation(
                                Ub[:], psU[:],
                                mybir.ActivationFunctionType.Copy)
                            preT = wp.tile([128, 128], BF16, tag="pre", bufs=2)
                            nc.vector.scalar_tensor_tensor(
                                preT[:], S_sb[:], ob, Ub[:],
                                mybir.AluOpType.mult, mybir.AluOpType.add)
                        else:
                            nc.tensor.matmul(psU[:], ident_sb[:],
                                             tprime[:, t * 128:(t + 1) * 128],
                                             start=True, stop=True)
                            preT = wp.tile([128, 128], BF16, tag="pre", bufs=2)
                            nc.scalar.activation(
                                preT[:], psU[:],
                                mybir.ActivationFunctionType.Copy)
                        if not last:
                            psR = pp.tile([128, 128], BF16, tag="pst", bufs=3)
                            nc.tensor.transpose(psR[:], preT[:], ident_sb[:])
                            rows = wp.tile([128, HID], BF16, tag="rows", bufs=3)
                            nc.scalar.activation(
                                rows[:], psR[:],
                                mybir.ActivationFunctionType.Relu,
                                scale=scol_sb[:, t:t + 1])
                            nc.sync.dma_start(
                                shard[t * 128:(t + 1) * 128, :], rows[:])
                        else:
                            reluT = wp.tile([128, 128], BF16, tag="relt", bufs=2)
                            nc.scalar.activation(
                                reluT[:], preT[:],
                                mybir.ActivationFunctionType.Relu)
                            pso = pp.tile([64, 128], F32, tag="ps", bufs=3)
                            nc.tensor.matmul(pso[:], wout_sb[:], reluT[:],
                                             start=True, stop=True)
                            oT = wp.tile([64, 128], BF16, tag="ub", bufs=3)
                            nc.scalar.activation(
                                oT[:], pso[:],
                                mybir.ActivationFunctionType.Copy)
                            psq = pp.tile([128, 64], BF16, tag="pst", bufs=3)
                            nc.tensor.transpose(psq[:], oT[:],
                                                ident_sb[:64, :64])
                            obt = wp.tile([128, 64], F32, tag="ob", bufs=3)
                            nc.vector.scalar_tensor_tensor(
                                obt[:], psq[:], acol_sb[:, t:t + 1], bout_sb[:],
                                mybir.AluOpType.mult, mybir.AluOpType.add)
                            r0 = t * 128
                            r1 = min(r0 + 128, NOWN)
                            if r1 > r0:
                                nc.sync.dma_start(out_ext[r0:r1, :],
                                                  obt[:r1 - r0, :])
                    # end tiles
                if not last:
                    ag(shard, tables[l + 1])

    nc.compile()
    return nc


def _host_inputs(inputs, pre):
    x = np.asarray(inputs["x"], np.float32)
    W_in = np.asarray(inputs["W_in"], np.float32)
    b_in = np.asarray(inputs["b_in"], np.float32)
    W_conv = np.asarray(inputs["W_conv"], np.float32)
    W_out = np.asarray(inputs["W_out"], np.float32)
    b_out = np.asarray(inputs["b_out"], np.float32)
    betas = np.array([math.log(THETA / (l + 1) + 1.0) for l in range(L)],
                     np.float32)

    win_w = np.zeros((128, KIN // 128, HID), np.float32)
    for k in range(KIN // 128):
        rows = W_in[k * 128:min((k + 1) * 128, IN_C)]
        win_w[:rows.shape[0], k, :] = rows
    wc_w = (W_conv * betas[:, None, None]).transpose(1, 0, 2).copy()  # [128,L,128]
    iota_w = np.tile(np.arange(128, dtype=np.float32)[None, :], (128, 1))
    ident_w = np.eye(128, dtype=np.float32)
    bout_w = np.tile(b_out[None, :], (128, 1)).astype(np.float32)
    bin_w = b_in.reshape(128, 1).astype(np.float32)

    xT_w = np.zeros((NCORES, 128, KIN // 128, NLOC), np.float32)
    xr = x.reshape(NCORES, NOWN, IN_C)
    for k in range(KIN // 128):
        c0, c1 = k * 128, min((k + 1) * 128, IN_C)
        xT_w[:, :c1 - c0, k, :NOWN] = xr[:, :, c0:c1].transpose(0, 2, 1)

    maps = []
    for c in range(NCORES):
        maps.append({
            "xT": xT_w[c].astype(BF), "win": win_w.astype(BF), "bin": bin_w,
            "wc": wc_w.astype(BF), "wout": W_out.astype(BF), "bout": bout_w.astype(BF),
            "iota": iota_w.astype(BF), "ident": ident_w.astype(BF),
            "dinvc": pre["dinv_col"][c], "acol": pre["acol"][c],
            "scol": pre["scol"][c], "hacol": pre["hacol"][c],
            "idx": pre["idx_w"][c], "dl": pre["dl_w"][c],
        })
    return maps


def kernel(**inputs):
    edge_index = np.asarray(inputs["edge_index"])
    key = hash(edge_index.tobytes())
    if key not in _cache:
        pre = _preprocess(edge_index)
        n_layers = int(os.environ.get("GCN_NL", L))
        max_tg = os.environ.get("GCN_MAXTG")
        nc = _build_program(pre, n_layers,
                            int(max_tg) if max_tg else None)
        _cache.clear()
        _cache[key] = (pre, nc)
    pre, nc = _cache[key]

    in_maps = _host_inputs(inputs, pre)
    trace = bool(os.environ.get("GCN_TRACE"))
    res = run_bass_kernel_spmd(nc, in_maps, core_ids=list(range(NCORES)),
                               trace=trace)
    LAST_PERF["exec_time_ns"] = res.exec_time_ns
    LAST_PERF["mean_exec_time_ns"] = res.mean_exec_time_ns
    LAST_PERF["trace"] = (res.instructions_and_trace or (None, None))[1]
    out = np.concatenate([res.results[c]["out"] for c in range(NCORES)], axis=0)
    return out.astype(np.float32)


# revision 6
# speedup vs baseline: 1.2340x; 1.1705x over previous
"""GCN2 (GCNII) message-passing kernel for 8 Trainium2 NeuronCores.

Strategy (1D node partition):
  - Nodes sharded 8 ways by id; each core owns 12500 nodes (padded to 12544).
  - Layer weights replicated; per-layer halo exchange as 4 chunked AllGathers
    of each core's h' shard into a replicated DRAM node table (bf16).
  - Table rows store h' = dinv * h in bf16 (256B rows) — halves gather and
    collective traffic vs fp32.
  - Aggregation (segment_sum over dst-sorted edges): dma_gather of 256B rows
    by src, then per-128-edge-chunk one-hot matmuls on the TensorEngine
    accumulating into PSUM (S^T[feat,dst] += G^T @ I01).
  - Indicators are PURE 0/1 and built in BULK: one vector tensor_tensor
    (is_equal) per tile-group over [128, nch*128] with broadcast APs
    (iota broadcast along chunks, dl broadcast along the 128 lane dim).
    Padded edge slots carry dl=200 which matches no lane. tensor_tensor
    never uses the DVE 2-port mode, so it does not take the SBUF port lock
    that serializes against GpSimd SWDGE descriptor generation.
  - The per-edge norm v = 0.9*dinv[dst] depends only on dst, so it commutes
    through the layer: with a_d = 0.9*dinv_d,
        h'_next_col_d = dinv_d*a_d * relu(U_d + T'_d)
    where U = (1-b)S + b W^T S  and  T' = ((1-b) + b W^T) h0a,
    h0a = 0.1*h0/a (resident bf16). The per-partition scales (0.9*dinv^2,
    1/(9*dinv)) are applied on the Scalar engine post-transpose.
  - Scalar engine (idle otherwise) does all PSUM->SBUF copies and relu.
"""

import math
import os
import sys

import numpy as np
import ml_dtypes

for _p in ("/opt/trn_rl_repo",):
    if _p not in sys.path and os.path.isdir(_p):
        sys.path.insert(0, _p)

import concourse.bacc as bacc
import concourse.mybir as mybir
import concourse.tile as tile
from concourse.bass_utils import run_bass_kernel_spmd

# ---------------- problem constants (hardcoded per contract) ----------------
N = 100_000
E = 1_600_000
IN_C = 500
HID = 128
OUT_C = 64
L = 8
ALPHA = 0.1
THETA = 0.5

NCORES = 8
NOWN = N // NCORES          # 12500 real nodes per core
NLOC = 12544                # padded to 98 * 128
NT = NLOC // 128            # 98 dst tiles per core
TGS = 4                     # dst tiles per gather group
KIN = 512                   # padded input channels
BANK = 32768                # int16-addressable rows per gather bank
AGC = 4096                  # shard rows per chunked AllGather (AG 0..2)
TROWS = 3 * BANK + NCORES * (NLOC - 3 * AGC)   # 100352 table rows
PAD_DL = 200.0              # dl sentinel for padded edge slots (matches no lane)

F32 = mybir.dt.float32
BF16 = mybir.dt.bfloat16
I16 = mybir.dt.int16
BF = ml_dtypes.bfloat16

_cache = {}

LAST_PERF = {}


def _row_of_node(n):
    """Table row of global node id(s) n (vectorized)."""
    c = n // NOWN
    i = n - c * NOWN
    q = np.minimum(i // AGC, 3)
    tail = NLOC - 3 * AGC  # 256
    return np.where(q < 3, q * BANK + c * AGC + (i - q * AGC),
                    3 * BANK + c * tail + (i - 3 * AGC))


def _preprocess(edge_index):
    """All graph-structure preprocessing on host (numpy)."""
    e0 = edge_index[0].astype(np.int64)
    e1 = edge_index[1].astype(np.int64)
    loop = np.arange(N, dtype=np.int64)
    src = np.concatenate([e0, loop])
    dst = np.concatenate([e1, loop])

    deg = np.bincount(dst, minlength=N).astype(np.float64)
    dinv = np.where(deg > 0, 1.0 / np.sqrt(deg), 0.0).astype(np.float32)

    owner = dst // NOWN
    ldst = dst - owner * NOWN
    t_arr = ldst // 128
    dl_arr = (ldst % 128).astype(np.float32)
    row = _row_of_node(src)
    b_arr = row // BANK
    bidx = (row % BANK).astype(np.int16)

    # group id and stable sort
    G = ((owner * NT + t_arr) * 4 + b_arr).astype(np.int64)
    order = np.argsort(G, kind="stable")
    Gs = G[order]
    counts = np.bincount(Gs, minlength=NCORES * NT * 4).reshape(NCORES, NT, 4)
    C = np.ceil(counts / 128).astype(np.int64).max(axis=0)  # [NT, 4]

    # chunk stream plan (identical for all cores)
    tg_tiles = [list(range(g, min(g + TGS, NT))) for g in range(0, NT, TGS)]
    tg_meta = []
    slot0 = np.zeros((NT, 4), np.int64)
    ch = 0
    for tiles in tg_tiles:
        tg_ch0 = ch
        banks = []
        for b in range(4):
            b_off = ch - tg_ch0
            ents = []
            for t in tiles:
                slot0[t, b] = ch * 128
                if C[t, b] > 0:
                    ents.append((t, ch - tg_ch0, int(C[t, b])))
                ch += C[t, b]
            banks.append((int(b_off), int(ch - tg_ch0 - b_off), ents))
        tg_meta.append(dict(ch0=int(tg_ch0), nch=int(ch - tg_ch0),
                            banks=banks, tiles=tiles))
    NCHUNK = int(ch)
    NSLOT = NCHUNK * 128

    # fill per-core flat arrays (vectorized scatter)
    gstart = np.zeros(NCORES * NT * 4 + 1, np.int64)
    np.cumsum(counts.reshape(-1), out=gstart[1:])
    rank = np.arange(len(Gs)) - gstart[Gs]
    core_of = Gs // (NT * 4)
    tb = Gs % (NT * 4)
    dest = core_of * NSLOT + slot0.reshape(-1)[tb] + rank

    idx_flat = np.zeros(NCORES * NSLOT, np.int16)
    dl_flat = np.full(NCORES * NSLOT, PAD_DL, np.float32)
    idx_flat[dest] = bidx[order]
    dl_flat[dest] = dl_arr[order]

    idx_flat = idx_flat.reshape(NCORES, NSLOT)
    dl_flat = dl_flat.reshape(NCORES, NSLOT)

    # device layouts
    idx_w = np.tile(
        idx_flat.reshape(NCORES, NSLOT // 16, 16).transpose(0, 2, 1), (1, 8, 1)
    ).copy()                                            # [c, 128, NSLOT/16]
    dl_w = dl_flat.reshape(NCORES, NCHUNK, 128).transpose(0, 2, 1)
    dl_w = dl_w.astype(BF).copy()                       # [c, 128, NCHUNK]

    dinv_loc = np.zeros((NCORES, NLOC), np.float32)
    dinv_loc[:, :NOWN] = dinv.reshape(NCORES, NOWN)
    dcol = dinv_loc.reshape(NCORES, NT, 128).transpose(0, 2, 1).copy()
    acol = (0.9 * dcol).astype(np.float32)
    scol = (0.9 * dcol * dcol).astype(np.float32)
    hacol = np.where(dcol > 0, 1.0 / (9.0 * np.maximum(dcol, 1e-30)),
                     0.0).astype(np.float32)

    return dict(tg_meta=tg_meta, NCHUNK=NCHUNK, NSLOT=NSLOT,
                idx_w=idx_w, dl_w=dl_w, dinv_col=dcol, acol=acol,
                scol=scol, hacol=hacol, counts=counts, C=C)


def _build_program(pre, n_layers=L, max_tg=None):
    nc = bacc.Bacc("TRN2", target_bir_lowering=False, debug=False,
                   num_devices=NCORES, num_swdge_queues=4)
    tg_meta = pre["tg_meta"]
    if max_tg is not None:
        tg_meta = tg_meta[:max_tg]
    gq = [0]  # round-robin SWDGE queue for gathers
    NCHUNK, NSLOT = pre["NCHUNK"], pre["NSLOT"]
    betas = [float(np.log(THETA / (l + 1) + 1.0)) for l in range(L)]
    skips = os.environ.get("GCN_SKIP", "").split(",")

    # ---- external inputs ----
    xT_in = nc.dram_tensor("xT", [128, KIN // 128, NLOC], BF16, kind="ExternalInput")
    win_in = nc.dram_tensor("win", [128, KIN // 128, HID], BF16, kind="ExternalInput")
    bin_in = nc.dram_tensor("bin", [128, 1], F32, kind="ExternalInput")
    wc_in = nc.dram_tensor("wc", [128, L, HID], BF16, kind="ExternalInput")
    wout_in = nc.dram_tensor("wout", [128, OUT_C], BF16, kind="ExternalInput")
    bout_in = nc.dram_tensor("bout", [128, OUT_C], BF16, kind="ExternalInput")
    iota_in = nc.dram_tensor("iota", [128, 128], BF16, kind="ExternalInput")
    ident_in = nc.dram_tensor("ident", [128, 128], BF16, kind="ExternalInput")
    dinv_in = nc.dram_tensor("dinvc", [128, NT], F32, kind="ExternalInput")
    acol_in = nc.dram_tensor("acol", [128, NT], F32, kind="ExternalInput")
    scol_in = nc.dram_tensor("scol", [128, NT], F32, kind="ExternalInput")
    hacol_in = nc.dram_tensor("hacol", [128, NT], F32, kind="ExternalInput")
    idx_in = nc.dram_tensor("idx", [128, NSLOT // 16], I16, kind="ExternalInput")
    dl_in = nc.dram_tensor("dl", [128, NCHUNK], BF16, kind="ExternalInput")
    out_ext = nc.dram_tensor("out", [NOWN, OUT_C], F32, kind="ExternalOutput")

    rg = [list(range(NCORES))]

    with tile.TileContext(nc, num_cores=NCORES) as tc:
        with (
            tc.tile_pool(name="const", bufs=1) as cpool,
            tc.tile_pool(name="dram", bufs=1, space="DRAM") as dram,
            tc.tile_pool(name="work", bufs=1) as wp,
            tc.tile_pool(name="psum", bufs=1, space="PSUM") as pp,
        ):
            # ---- resident constants ----
            win_sb = cpool.tile([128, KIN // 128, HID], BF16)
            bin_sb = cpool.tile([128, 1], F32)
            wc_sb = cpool.tile([128, L, HID], BF16)
            wout_sb = cpool.tile([128, OUT_C], BF16)
            bout_sb = cpool.tile([128, OUT_C], BF16)
            iota_sb = cpool.tile([128, 128], BF16)
            ident_sb = cpool.tile([128, 128], BF16)
            dinv_sb = cpool.tile([128, NT], F32)
            acol_sb = cpool.tile([128, NT], F32)
            scol_sb = cpool.tile([128, NT], F32)
            hacol_sb = cpool.tile([128, NT], F32)
            h0a = cpool.tile([128, NLOC], BF16)     # 0.1*h0/a resident
            tprime = cpool.tile([128, NLOC], BF16)  # per-layer T' buffer
            for sb_t, ext in ((win_sb, win_in), (bin_sb, bin_in), (wc_sb, wc_in),
                              (wout_sb, wout_in), (bout_sb, bout_in),
                              (iota_sb, iota_in), (ident_sb, ident_in),
                              (dinv_sb, dinv_in), (acol_sb, acol_in),
                              (scol_sb, scol_in), (hacol_sb, hacol_in)):
                nc.sync.dma_start(sb_t[:], ext[:])

            # one Shared table tensor per (allgather round, bank)
            NBROWS = [BANK, BANK, BANK, TROWS - 3 * BANK]
            tables = [
                [dram.tile([NBROWS[b], HID], BF16, addr_space="Shared",
                           name=f"table{r}_{b}") for b in range(4)]
                for r in range(L)
            ]
            shards = [
                dram.tile([NLOC, HID], BF16, name=f"shard{i}") for i in range(2)
            ]

            def ag(shard, tb):
                """Chunked AllGather shard -> table (4 calls)."""
                bounds = [(0, AGC), (AGC, 2 * AGC), (2 * AGC, 3 * AGC),
                          (3 * AGC, NLOC)]
                for b, (r0, r1) in enumerate(bounds):
                    nc.gpsimd.collective_compute(
                        "AllGather", mybir.AluOpType.bypass, replica_groups=rg,
                        ins=[shard[r0:r1, :].opt()],
                        outs=[tb[b][:, :].opt()],
                    )

            # ========= input layer: h0 = relu(x @ W_in + b_in) =========
            # Produces: shard0 rows (dinv*h0, bf16) and resident h0a
            # (= 0.1*h0/a, bf16, [feat, node]).
            shard0 = shards[0]
            for g0 in range(0, NLOC, 512):
                w = min(512, NLOC - g0)
                psin = pp.tile([128, 512], F32, tag="big", bufs=2)
                for k in range(KIN // 128):
                    xt = wp.tile([128, 512], BF16, tag="xt", bufs=3)
                    nc.sync.dma_start(xt[:, :w], xT_in[:, k, g0:g0 + w])
                    nc.tensor.matmul(psin[:, :w], win_sb[:, k, :], xt[:, :w],
                                     start=(k == 0), stop=(k == KIN // 128 - 1))
                h0T = wp.tile([128, 512], BF16, tag="h0t", bufs=2)
                nc.scalar.activation(h0T[:, :w], psin[:, :w],
                                     mybir.ActivationFunctionType.Relu,
                                     bias=bin_sb[:, 0:1])
                for j in range(0, w, 128):
                    t = (g0 + j) // 128
                    pst = pp.tile([128, 128], BF16, tag="pst", bufs=3)
                    nc.tensor.transpose(pst[:], h0T[:, j:j + 128], ident_sb[:])
                    rows0 = wp.tile([128, HID], BF16, tag="rows", bufs=3)
                    nc.scalar.activation(rows0[:], pst[:],
                                         mybir.ActivationFunctionType.Copy,
                                         scale=dinv_sb[:, t:t + 1])
                    nc.sync.dma_start(shard0[t * 128:(t + 1) * 128, :], rows0[:])
                    h0ar = wp.tile([128, HID], BF16, tag="h0ar", bufs=2)
                    nc.scalar.activation(h0ar[:], pst[:],
                                         mybir.ActivationFunctionType.Copy,
                                         scale=hacol_sb[:, t:t + 1])
                    ps2 = pp.tile([128, 128], BF16, tag="ps", bufs=3)
                    nc.tensor.transpose(ps2[:], h0ar[:], ident_sb[:])
                    nc.scalar.activation(h0a[:, t * 128:(t + 1) * 128], ps2[:],
                                         mybir.ActivationFunctionType.Copy)
            ag(shard0, tables[0])

            # ========================= L layers =============================
            for l in range(n_layers):
                t_in = tables[l]
                shard = shards[(l + 1) % 2]
                last = l == L - 1
                ob = 1.0 - betas[l]

                # T' = (1-b)*h0a + b*(W^T h0a)  (wc already holds b*W)
                for g0 in range(0, NLOC, 512):
                    w = min(512, NLOC - g0)
                    psT = pp.tile([128, 512], F32, tag="big", bufs=2)
                    nc.tensor.matmul(psT[:, :w], wc_sb[:, l, :],
                                     h0a[:, g0:g0 + w], start=True, stop=True)
                    tpb = wp.tile([128, 512], BF16, tag="tpb", bufs=2)
                    nc.scalar.activation(tpb[:, :w], psT[:, :w],
                                         mybir.ActivationFunctionType.Copy)
                    nc.vector.scalar_tensor_tensor(
                        tprime[:, g0:g0 + w], h0a[:, g0:g0 + w], ob,
                        tpb[:, :w], mybir.AluOpType.mult, mybir.AluOpType.add)

                for tg in tg_meta:
                    nch = tg["nch"]
                    ch0 = tg["ch0"]
                    idx_t = wp.tile([128, nch * 8], I16, tag="idxs", bufs=3)
                    nc.sync.dma_start(idx_t[:], idx_in[:, ch0 * 8:(ch0 + nch) * 8])
                    dl_t = wp.tile([128, nch], BF16, tag="dlt", bufs=3)
                    nc.sync.dma_start(dl_t[:], dl_in[:, ch0:ch0 + nch])

                    # bulk 0/1 indicator build: one DVE tensor_tensor per tg
                    ind = wp.tile([128, nch, 128], BF16, tag="ind", bufs=2)
                    nc.vector.tensor_tensor(
                        ind[:, :, :],
                        iota_sb[:, :].unsqueeze(1).broadcast_to([128, nch, 128]),
                        dl_t[:, :].unsqueeze(2).broadcast_to([128, nch, 128]),
                        mybir.AluOpType.is_equal)

                    gbuf = wp.tile([128, nch, HID], BF16, tag="g", bufs=2)
                    per_tile = {t: [] for t in tg["tiles"]}
                    for b, (b_off, b_nch, ents) in enumerate(tg["banks"]):
                        if b_nch == 0:
                            continue
                        if "gather" not in skips:
                            # single_packet coalesces each call into one
                            # descriptor chain per SDMA engine (reads
                            # pipeline), but the chain must stay <= 64
                            # descriptors -> <= 8 chunks (1024 rows) per call.
                            for s0 in range(b_off, b_off + b_nch, 8):
                                sn = min(8, b_off + b_nch - s0)
                                nc.gpsimd.dma_gather(
                                    gbuf[:, s0:s0 + sn, :],
                                    t_in[b][:, :],
                                    idx_t[:, s0 * 8:(s0 + sn) * 8],
                                    sn * 128, sn * 128, HID,
                                    single_packet=True,
                                    queue_num=gq[0] % 4,
                                )
                                gq[0] += 1
                        for (t, pos, cnt) in ents:
                            per_tile[t].append((pos, cnt))

                    for t in tg["tiles"]:
                        chunks = [p for (pos, cnt) in per_tile[t]
                                  for p in range(pos, pos + cnt)]
                        if "mm" in skips:
                            chunks = []
                        psS = pp.tile([128, 128], F32, tag="ps", bufs=3)
                        for k, p in enumerate(chunks):
                            nc.tensor.matmul(psS[:], gbuf[:, p, :], ind[:, p, :],
                                             start=(k == 0),
                                             stop=(k == len(chunks) - 1))
                        # U path: psU = b*W^T S + T'
                        psU = pp.tile([128, 128], F32, tag="pst", bufs=3)
                        if chunks:
                            S_sb = wp.tile([128, 128], BF16, tag="ssb", bufs=3)
                            nc.scalar.activation(
                                S_sb[:], psS[:],
                                mybir.ActivationFunctionType.Copy)
                            nc.tensor.matmul(psU[:], wc_sb[:, l, :], S_sb[:],
                                             start=True, stop=False)
                            nc.tensor.matmul(psU[:], ident_sb[:],
                                             tprime[:, t * 128:(t + 1) * 128],
                                             start=False, stop=True)
                            Ub = wp.tile([128, 128], BF16, tag="ub", bufs=3)
                            nc.scalar.activation(
                                Ub[:], psU[:],
                                mybir.ActivationFunctionType.Copy)
                            preT = wp.tile([128, 128], BF16, tag="pre", bufs=2)
                            nc.vector.scalar_tensor_tensor(
                                preT[:], S_sb[:], ob, Ub[:],
                                mybir.AluOpType.mult, mybir.AluOpType.add)
                        else:
                            nc.tensor.matmul(psU[:], ident_sb[:],
                                             tprime[:, t * 128:(t + 1) * 128],
                                             start=True, stop=True)
                            preT = wp.tile([128, 128], BF16, tag="pre", bufs=2)
                            nc.scalar.activation(
                                preT[:], psU[:],
                                mybir.ActivationFunctionType.Copy)
                        if not last:
                            psR = pp.tile([128, 128], BF16, tag="pst", bufs=3)
                            nc.tensor.transpose(psR[:], preT[:], ident_sb[:])
                            rows = wp.tile([128, HID], BF16, tag="rows", bufs=3)
                            nc.scalar.activation(
                                rows[:], psR[:],
                                mybir.ActivationFunctionType.Relu,
                                scale=scol_sb[:, t:t + 1])
                            nc.sync.dma_start(
                                shard[t * 128:(t + 1) * 128, :], rows[:])
                        else:
                            reluT = wp.tile([128, 128], BF16, tag="relt", bufs=2)
                            nc.scalar.activation(
                                reluT[:], preT[:],
                                mybir.ActivationFunctionType.Relu)
                            pso = pp.tile([64, 128], F32, tag="ps", bufs=3)
                            nc.tensor.matmul(pso[:], wout_sb[:], reluT[:],
                                             start=True, stop=True)
                            oT = wp.tile([64, 128], BF16, tag="ub", bufs=3)
                            nc.scalar.activation(
                                oT[:], pso[:],
                                mybir.ActivationFunctionType.Copy)
                            psq = pp.tile([128, 64], BF16, tag="pst", bufs=3)
                            nc.tensor.transpose(psq[:], oT[:],
                                                ident_sb[:64, :64])
                            obt = wp.tile([128, 64], F32, tag="ob", bufs=3)
                            nc.vector.scalar_tensor_tensor(
                                obt[:], psq[:], acol_sb[:, t:t + 1], bout_sb[:],
                                mybir.AluOpType.mult, mybir.AluOpType.add)
                            r0 = t * 128
                            r1 = min(r0 + 128, NOWN)
                            if r1 > r0:
                                nc.sync.dma_start(out_ext[r0:r1, :],
                                                  obt[:r1 - r0, :])
                    # end tiles
                if not last:
                    ag(shard, tables[l + 1])

    nc.compile()
    return nc


def _host_inputs(inputs, pre):
    x = np.asarray(inputs["x"], np.float32)
    W_in = np.asarray(inputs["W_in"], np.float32)
    b_in = np.asarray(inputs["b_in"], np.float32)
    W_conv = np.asarray(inputs["W_conv"], np.float32)
    W_out = np.asarray(inputs["W_out"], np.float32)
    b_out = np.asarray(inputs["b_out"], np.float32)
    betas = np.array([math.log(THETA / (l + 1) + 1.0) for l in range(L)],
                     np.float32)

    win_w = np.zeros((128, KIN // 128, HID), np.float32)
    for k in range(KIN // 128):
        rows = W_in[k * 128:min((k + 1) * 128, IN_C)]
        win_w[:rows.shape[0], k, :] = rows
    wc_w = (W_conv * betas[:, None, None]).transpose(1, 0, 2).copy()  # [128,L,128]
    iota_w = np.tile(np.arange(128, dtype=np.float32)[None, :], (128, 1))
    ident_w = np.eye(128, dtype=np.float32)
    bout_w = np.tile(b_out[None, :], (128, 1)).astype(np.float32)
    bin_w = b_in.reshape(128, 1).astype(np.float32)

    xT_w = np.zeros((NCORES, 128, KIN // 128, NLOC), np.float32)
    xr = x.reshape(NCORES, NOWN, IN_C)
    for k in range(KIN // 128):
        c0, c1 = k * 128, min((k + 1) * 128, IN_C)
        xT_w[:, :c1 - c0, k, :NOWN] = xr[:, :, c0:c1].transpose(0, 2, 1)

    maps = []
    for c in range(NCORES):
        maps.append({
            "xT": xT_w[c].astype(BF), "win": win_w.astype(BF), "bin": bin_w,
            "wc": wc_w.astype(BF), "wout": W_out.astype(BF), "bout": bout_w.astype(BF),
            "iota": iota_w.astype(BF), "ident": ident_w.astype(BF),
            "dinvc": pre["dinv_col"][c], "acol": pre["acol"][c],
            "scol": pre["scol"][c], "hacol": pre["hacol"][c],
            "idx": pre["idx_w"][c], "dl": pre["dl_w"][c],
        })
    return maps


def kernel(**inputs):
    edge_index = np.asarray(inputs["edge_index"])
    key = hash(edge_index.tobytes())
    if key not in _cache:
        pre = _preprocess(edge_index)
        n_layers = int(os.environ.get("GCN_NL", L))
        max_tg = os.environ.get("GCN_MAXTG")
        nc = _build_program(pre, n_layers,
                            int(max_tg) if max_tg else None)
        _cache.clear()
        _cache[key] = (pre, nc)
    pre, nc = _cache[key]

    in_maps = _host_inputs(inputs, pre)
    trace = bool(os.environ.get("GCN_TRACE"))
    res = run_bass_kernel_spmd(nc, in_maps, core_ids=list(range(NCORES)),
                               trace=trace)
    LAST_PERF["exec_time_ns"] = res.exec_time_ns
    LAST_PERF["mean_exec_time_ns"] = res.mean_exec_time_ns
    LAST_PERF["trace"] = (res.instructions_and_trace or (None, None))[1]
    out = np.concatenate([res.results[c]["out"] for c in range(NCORES)], axis=0)
    return out.astype(np.float32)


# revision 11
# speedup vs baseline: 2.5070x; 2.0316x over previous
"""GCN2 (GCNII) message-passing kernel for 8 Trainium2 NeuronCores.

Strategy (1D node partition):
  - Nodes sharded 8 ways by id; each core owns 12500 nodes (padded to 12544).
  - Layer weights replicated; per-layer halo exchange as 4 chunked AllGathers
    of each core's h' shard into a replicated DRAM node table (bf16).
  - Table rows store h' = dinv * h in bf16 (256B rows) — halves gather and
    collective traffic vs fp32.
  - Aggregation (segment_sum over dst-sorted edges): dma_gather of 256B rows
    by src, then per-128-edge-chunk one-hot matmuls on the TensorEngine
    accumulating into PSUM (S^T[feat,dst] += G^T @ I01).
  - Indicators are PURE 0/1 and built in BULK: one vector tensor_tensor
    (is_equal) per tile-group over [128, nch*128] with broadcast APs
    (iota broadcast along chunks, dl broadcast along the 128 lane dim).
    Padded edge slots carry dl=200 which matches no lane. tensor_tensor
    never uses the DVE 2-port mode, so it does not take the SBUF port lock
    that serializes against GpSimd SWDGE descriptor generation.
  - The per-edge norm v = 0.9*dinv[dst] depends only on dst, so it commutes
    through the layer: with a_d = 0.9*dinv_d,
        h'_next_col_d = dinv_d*a_d * relu(U_d + T'_d)
    where U = (1-b)S + b W^T S  and  T' = ((1-b) + b W^T) h0a,
    h0a = 0.1*h0/a (resident bf16). The per-partition scales (0.9*dinv^2,
    1/(9*dinv)) are applied on the Scalar engine post-transpose.
  - Scalar engine (idle otherwise) does all PSUM->SBUF copies and relu.
"""

import math
import os
import sys

import numpy as np
import ml_dtypes

for _p in ("/opt/trn_rl_repo",):
    if _p not in sys.path and os.path.isdir(_p):
        sys.path.insert(0, _p)

import concourse.bacc as bacc
import concourse.mybir as mybir
import concourse.tile as tile
from concourse.bass_utils import run_bass_kernel_spmd

# ---------------- problem constants (hardcoded per contract) ----------------
N = 100_000
E = 1_600_000
IN_C = 500
HID = 128
OUT_C = 64
L = 8
ALPHA = 0.1
THETA = 0.5

NCORES = 8
NOWN = N // NCORES          # 12500 real nodes per core
NLOC = 12544                # padded to 98 * 128
NT = NLOC // 128            # 98 dst tiles per core
TGS = 4                     # dst tiles per gather group
KIN = 512                   # padded input channels
BANK = 32768                # int16-addressable rows per gather bank
AGC = 4096                  # shard rows per chunked AllGather (AG 0..2)
TROWS = 3 * BANK + NCORES * (NLOC - 3 * AGC)   # 100352 table rows
PAD_DL = 1024.0             # dl sentinel for padded edge slots (matches no lane)

F32 = mybir.dt.float32
BF16 = mybir.dt.bfloat16
F16 = mybir.dt.float16
I16 = mybir.dt.int16
BF = ml_dtypes.bfloat16

_cache = {}

LAST_PERF = {}


def _row_of_node(n):
    """Table row of global node id(s) n (vectorized)."""
    c = n // NOWN
    i = n - c * NOWN
    q = np.minimum(i // AGC, 3)
    tail = NLOC - 3 * AGC  # 256
    return np.where(q < 3, q * BANK + c * AGC + (i - q * AGC),
                    3 * BANK + c * tail + (i - 3 * AGC))


def _preprocess(edge_index):
    """All graph-structure preprocessing on host (numpy).

    Edge slots are packed at (tilegroup, bank) granularity: within one
    (core, tg, bank) segment, edges are sorted by dst tile and placed
    contiguously, so padding only appears at segment tails and is marked
    idx=-1 (the gather ucode strips trailing negatives — those rows are
    never fetched). dl holds the tg-relative dst (0..TGS*128), compared
    against per-tile shifted iotas on device; dl/iota use fp16 so all
    values are exact.
    """
    e0 = edge_index[0].astype(np.int64)
    e1 = edge_index[1].astype(np.int64)
    loop = np.arange(N, dtype=np.int64)
    src = np.concatenate([e0, loop])
    dst = np.concatenate([e1, loop])

    deg = np.bincount(dst, minlength=N).astype(np.float64)
    dinv = np.where(deg > 0, 1.0 / np.sqrt(deg), 0.0).astype(np.float32)

    owner = dst // NOWN
    ldst = dst - owner * NOWN
    t_arr = ldst // 128
    row = _row_of_node(src)
    b_arr = row // BANK
    bidx = (row % BANK).astype(np.int16)

    TGN = (NT + TGS - 1) // TGS          # 25 tilegroups
    tgid = t_arr // TGS
    # segment id (core, tg, bank); sort by segment then dst tile
    G = ((owner * TGN + tgid) * 4 + b_arr).astype(np.int64)
    order = np.lexsort((t_arr, G))
    Gs = G[order]
    ts = t_arr[order]
    # per-(core,tg,bank) totals and per-(core,tile,bank) totals
    seg_counts = np.bincount(Gs, minlength=NCORES * TGN * 4)
    seg_counts = seg_counts.reshape(NCORES, TGN, 4)
    tb_counts = np.bincount((owner * NT + t_arr) * 4 + b_arr,
                            minlength=NCORES * NT * 4)
    tb_counts = tb_counts.reshape(NCORES, NT, 4)
    C = np.ceil(seg_counts / 128).astype(np.int64).max(axis=0)  # [TGN, 4]

    # chunk stream plan (identical for all cores)
    tg_tiles = [list(range(g, min(g + TGS, NT))) for g in range(0, NT, TGS)]
    tg_meta = []
    seg_ch0 = np.zeros((TGN, 4), np.int64)   # absolute chunk of segment start
    ch = 0
    for tg, tiles in enumerate(tg_tiles):
        tg_ch0 = ch
        banks = []
        for b in range(4):
            seg_ch0[tg, b] = ch
            fill_lo = int(seg_counts[:, tg, b].min()) // 128
            banks.append((int(ch - tg_ch0), int(C[tg, b]), fill_lo))
            ch += C[tg, b]
        # per-tile chunk ranges (union over cores), relative to tg_ch0
        tinfo = []
        for q, t in enumerate(tiles):
            spans = []
            for b in range(4):
                if C[tg, b] == 0:
                    continue
                # per-core slot offsets of this tile within the segment
                starts, ends = [], []
                for c in range(NCORES):
                    off = int(tb_counts[c, tiles[0]:t, b].sum())
                    cnt = int(tb_counts[c, t, b])
                    if cnt > 0:
                        starts.append(off)
                        ends.append(off + cnt)
                if not starts:
                    continue
                r0 = min(starts) // 128
                r1 = -(-max(ends) // 128)
                base = int(seg_ch0[tg, b] - tg_ch0)
                spans.append((b, base + r0, base + r1))
            tinfo.append((t, q, spans))
        tg_meta.append(dict(ch0=int(tg_ch0), nch=int(ch - tg_ch0),
                            banks=banks, tinfo=tinfo))
    NCHUNK = int(ch)
    NSLOT = NCHUNK * 128

    # fill per-core flat arrays (vectorized scatter)
    gstart = np.zeros(NCORES * TGN * 4 + 1, np.int64)
    np.cumsum(seg_counts.reshape(-1), out=gstart[1:])
    rank = np.arange(len(Gs)) - gstart[Gs]
    core_of = Gs // (TGN * 4)
    tgb = Gs % (TGN * 4)
    dest = core_of * NSLOT + seg_ch0.reshape(-1)[tgb] * 128 + rank

    dl_val = (ldst - (tgid * TGS) * 128).astype(np.float32)  # tg-relative dst

    idx_flat = np.zeros(NCORES * NSLOT, np.int16)
    dl_flat = np.full(NCORES * NSLOT, PAD_DL, np.float32)
    idx_flat[dest] = bidx[order]
    dl_flat[dest] = dl_val[order]

    idx_flat = idx_flat.reshape(NCORES, NSLOT)
    dl_flat = dl_flat.reshape(NCORES, NSLOT)

    # device layouts
    idx_w = np.tile(
        idx_flat.reshape(NCORES, NSLOT // 16, 16).transpose(0, 2, 1), (1, 8, 1)
    ).copy()                                            # [c, 128, NSLOT/16]
    dl_w = dl_flat.reshape(NCORES, NCHUNK, 128).transpose(0, 2, 1)
    dl_w = dl_w.astype(np.float16).copy()               # [c, 128, NCHUNK]

    dinv_loc = np.zeros((NCORES, NLOC), np.float32)
    dinv_loc[:, :NOWN] = dinv.reshape(NCORES, NOWN)
    dcol = dinv_loc.reshape(NCORES, NT, 128).transpose(0, 2, 1).copy()
    acol = (0.9 * dcol).astype(np.float32)
    scol = (0.9 * dcol * dcol).astype(np.float32)
    hacol = np.where(dcol > 0, 1.0 / (9.0 * np.maximum(dcol, 1e-30)),
                     0.0).astype(np.float32)

    return dict(tg_meta=tg_meta, NCHUNK=NCHUNK, NSLOT=NSLOT,
                idx_w=idx_w, dl_w=dl_w, dinv_col=dcol, acol=acol,
                scol=scol, hacol=hacol, counts=tb_counts, C=C)


def _build_program(pre, n_layers=L, max_tg=None):
    nc = bacc.Bacc("TRN2", target_bir_lowering=False, debug=False,
                   num_devices=NCORES, num_swdge_queues=4)
    tg_meta = pre["tg_meta"]
    if max_tg is not None:
        tg_meta = tg_meta[:max_tg]
    gq = [0]  # round-robin SWDGE queue for gathers
    NCHUNK, NSLOT = pre["NCHUNK"], pre["NSLOT"]
    betas = [float(np.log(THETA / (l + 1) + 1.0)) for l in range(L)]
    skips = os.environ.get("GCN_SKIP", "").split(",")

    # ---- external inputs ----
    xT_in = nc.dram_tensor("xT", [128, KIN // 128, NLOC], BF16, kind="ExternalInput")
    win_in = nc.dram_tensor("win", [128, KIN // 128, HID], BF16, kind="ExternalInput")
    bin_in = nc.dram_tensor("bin", [128, 1], F32, kind="ExternalInput")
    wc_in = nc.dram_tensor("wc", [128, L, HID], BF16, kind="ExternalInput")
    wout_in = nc.dram_tensor("wout", [128, OUT_C], BF16, kind="ExternalInput")
    bout_in = nc.dram_tensor("bout", [128, OUT_C], BF16, kind="ExternalInput")
    iota_in = nc.dram_tensor("iota", [128, TGS * 128], F16, kind="ExternalInput")
    ident_in = nc.dram_tensor("ident", [128, 128], BF16, kind="ExternalInput")
    dinv_in = nc.dram_tensor("dinvc", [128, NT], F32, kind="ExternalInput")
    acol_in = nc.dram_tensor("acol", [128, NT], F32, kind="ExternalInput")
    scol_in = nc.dram_tensor("scol", [128, NT], F32, kind="ExternalInput")
    hacol_in = nc.dram_tensor("hacol", [128, NT], F32, kind="ExternalInput")
    idx_in = nc.dram_tensor("idx", [128, NSLOT // 16], I16, kind="ExternalInput")
    dl_in = nc.dram_tensor("dl", [128, NCHUNK], F16, kind="ExternalInput")
    out_ext = nc.dram_tensor("out", [NOWN, OUT_C], F32, kind="ExternalOutput")

    rg = [list(range(NCORES))]

    with tile.TileContext(nc, num_cores=NCORES) as tc:
        with (
            tc.tile_pool(name="const", bufs=1) as cpool,
            tc.tile_pool(name="dram", bufs=1, space="DRAM") as dram,
            tc.tile_pool(name="work", bufs=1) as wp,
            tc.tile_pool(name="psum", bufs=1, space="PSUM") as pp,
        ):
            # ---- resident constants ----
            win_sb = cpool.tile([128, KIN // 128, HID], BF16)
            bin_sb = cpool.tile([128, 1], F32)
            wc_sb = cpool.tile([128, L, HID], BF16)
            wout_sb = cpool.tile([128, OUT_C], BF16)
            bout_sb = cpool.tile([128, OUT_C], BF16)
            iota_sb = cpool.tile([128, TGS * 128], F16)
            ident_sb = cpool.tile([128, 128], BF16)
            dinv_sb = cpool.tile([128, NT], F32)
            acol_sb = cpool.tile([128, NT], F32)
            scol_sb = cpool.tile([128, NT], F32)
            hacol_sb = cpool.tile([128, NT], F32)
            h0a = cpool.tile([128, NLOC], BF16)     # 0.1*h0/a resident
            tprime = cpool.tile([128, NLOC], BF16)  # per-layer T' buffer
            for sb_t, ext in ((win_sb, win_in), (bin_sb, bin_in), (wc_sb, wc_in),
                              (wout_sb, wout_in), (bout_sb, bout_in),
                              (iota_sb, iota_in), (ident_sb, ident_in),
                              (dinv_sb, dinv_in), (acol_sb, acol_in),
                              (scol_sb, scol_in), (hacol_sb, hacol_in)):
                nc.sync.dma_start(sb_t[:], ext[:])

            # one Shared table tensor per (allgather round, bank)
            NBROWS = [BANK, BANK, BANK, TROWS - 3 * BANK]
            tables = [
                [dram.tile([NBROWS[b], HID], BF16, addr_space="Shared",
                           name=f"table{r}_{b}") for b in range(4)]
                for r in range(L)
            ]
            shards = [
                dram.tile([NLOC, HID], BF16, name=f"shard{i}") for i in range(2)
            ]

            def ag(shard, tb):
                """Chunked AllGather shard -> table (4 calls)."""
                bounds = [(0, AGC), (AGC, 2 * AGC), (2 * AGC, 3 * AGC),
                          (3 * AGC, NLOC)]
                for b, (r0, r1) in enumerate(bounds):
                    nc.gpsimd.collective_compute(
                        "AllGather", mybir.AluOpType.bypass, replica_groups=rg,
                        ins=[shard[r0:r1, :].opt()],
                        outs=[tb[b][:, :].opt()],
                    )

            # ========= input layer: h0 = relu(x @ W_in + b_in) =========
            # Produces: shard0 rows (dinv*h0, bf16) and resident h0a
            # (= 0.1*h0/a, bf16, [feat, node]).
            shard0 = shards[0]
            for g0 in range(0, NLOC, 512):
                w = min(512, NLOC - g0)
                psin = pp.tile([128, 512], F32, tag="big", bufs=2)
                for k in range(KIN // 128):
                    xt = wp.tile([128, 512], BF16, tag="xt", bufs=3)
                    nc.sync.dma_start(xt[:, :w], xT_in[:, k, g0:g0 + w])
                    nc.tensor.matmul(psin[:, :w], win_sb[:, k, :], xt[:, :w],
                                     start=(k == 0), stop=(k == KIN // 128 - 1))
                h0T = wp.tile([128, 512], BF16, tag="h0t", bufs=2)
                nc.scalar.activation(h0T[:, :w], psin[:, :w],
                                     mybir.ActivationFunctionType.Relu,
                                     bias=bin_sb[:, 0:1])
                for j in range(0, w, 128):
                    t = (g0 + j) // 128
                    pst = pp.tile([128, 128], BF16, tag="pst", bufs=3)
                    nc.tensor.transpose(pst[:], h0T[:, j:j + 128], ident_sb[:])
                    rows0 = wp.tile([128, HID], BF16, tag="rows", bufs=3)
                    nc.scalar.activation(rows0[:], pst[:],
                                         mybir.ActivationFunctionType.Copy,
                                         scale=dinv_sb[:, t:t + 1])
                    nc.sync.dma_start(shard0[t * 128:(t + 1) * 128, :], rows0[:])
                    h0ar = wp.tile([128, HID], BF16, tag="h0ar", bufs=2)
                    nc.scalar.activation(h0ar[:], pst[:],
                                         mybir.ActivationFunctionType.Copy,
                                         scale=hacol_sb[:, t:t + 1])
                    ps2 = pp.tile([128, 128], BF16, tag="ps", bufs=3)
                    nc.tensor.transpose(ps2[:], h0ar[:], ident_sb[:])
                    nc.scalar.activation(h0a[:, t * 128:(t + 1) * 128], ps2[:],
                                         mybir.ActivationFunctionType.Copy)
            ag(shard0, tables[0])

            # ========================= L layers =============================
            for l in range(n_layers):
                t_in = tables[l]
                shard = shards[(l + 1) % 2]
                last = l == L - 1
                ob = 1.0 - betas[l]

                # T' = (1-b)*h0a + b*(W^T h0a)  (wc already holds b*W)
                for g0 in range(0, NLOC, 512):
                    w = min(512, NLOC - g0)
                    psT = pp.tile([128, 512], F32, tag="big", bufs=2)
                    nc.tensor.matmul(psT[:, :w], wc_sb[:, l, :],
                                     h0a[:, g0:g0 + w], start=True, stop=True)
                    tpb = wp.tile([128, 512], BF16, tag="tpb", bufs=2)
                    nc.scalar.activation(tpb[:, :w], psT[:, :w],
                                         mybir.ActivationFunctionType.Copy)
                    nc.vector.scalar_tensor_tensor(
                        tprime[:, g0:g0 + w], h0a[:, g0:g0 + w], ob,
                        tpb[:, :w], mybir.AluOpType.mult, mybir.AluOpType.add)

                for tg in tg_meta:
                    nch = tg["nch"]
                    ch0 = tg["ch0"]
                    idx_t = wp.tile([128, nch * 8], I16, tag="idxs", bufs=3)
                    nc.sync.dma_start(idx_t[:], idx_in[:, ch0 * 8:(ch0 + nch) * 8])
                    dl_t = wp.tile([128, nch], F16, tag="dlt", bufs=3)
                    nc.sync.dma_start(dl_t[:], dl_in[:, ch0:ch0 + nch])

                    gbuf = wp.tile([128, nch, HID], BF16, tag="g", bufs=3)
                    for b, (b_off, b_nch, fill_lo) in enumerate(tg["banks"]):
                        if b_nch == 0:
                            continue
                        if "gather" not in skips:
                            # single_packet coalesces each call into one
                            # descriptor chain per SDMA engine (reads
                            # pipeline), but the chain must stay <= 64
                            # descriptors -> <= 8 chunks (1024 rows) per call.
                            for s0 in range(b_off, b_off + b_nch, 8):
                                sn = min(8, b_off + b_nch - s0)
                                nc.gpsimd.dma_gather(
                                    gbuf[:, s0:s0 + sn, :],
                                    t_in[b][:, :],
                                    idx_t[:, s0 * 8:(s0 + sn) * 8],
                                    sn * 128, sn * 128, HID,
                                    single_packet=True,
                                    queue_num=gq[0] % 4,
                                )
                                gq[0] += 1

                    for (t, q, spans) in tg["tinfo"]:
                        # per-(tile,bank) 0/1 indicator builds over the
                        # tile's chunk sub-range, comparing tg-relative dl
                        # against this tile's shifted iota slice.
                        chunks = []
                        if "mm" not in skips:
                            for (b, r0, r1) in spans:
                                span = r1 - r0
                                it = wp.tile([128, span, 128], BF16,
                                             tag="ind", bufs=8)
                                nc.vector.tensor_tensor(
                                    it[:, :, :],
                                    iota_sb[:, q * 128:(q + 1) * 128]
                                    .unsqueeze(1)
                                    .broadcast_to([128, span, 128]),
                                    dl_t[:, r0:r1].unsqueeze(2)
                                    .broadcast_to([128, span, 128]),
                                    mybir.AluOpType.is_equal)
                                for rel in range(span):
                                    chunks.append((it, rel, r0 + rel))
                        psS = pp.tile([128, 128], F32, tag="ps", bufs=3)
                        for k, (it, rel, p) in enumerate(chunks):
                            nc.tensor.matmul(psS[:], gbuf[:, p, :],
                                             it[:, rel, :],
                                             start=(k == 0),
                                             stop=(k == len(chunks) - 1))
                        # U path: psU = b*W^T S + T'
                        psU = pp.tile([128, 128], F32, tag="pst", bufs=3)
                        if chunks:
                            S_sb = wp.tile([128, 128], BF16, tag="ssb", bufs=3)
                            nc.scalar.activation(
                                S_sb[:], psS[:],
                                mybir.ActivationFunctionType.Copy)
                            nc.tensor.matmul(psU[:], wc_sb[:, l, :], S_sb[:],
                                             start=True, stop=False)
                            nc.tensor.matmul(psU[:], ident_sb[:],
                                             tprime[:, t * 128:(t + 1) * 128],
                                             start=False, stop=True)
                            Ub = wp.tile([128, 128], BF16, tag="ub", bufs=3)
                            nc.scalar.activation(
                                Ub[:], psU[:],
                                mybir.ActivationFunctionType.Copy)
                            preT = wp.tile([128, 128], BF16, tag="pre", bufs=2)
                            nc.vector.scalar_tensor_tensor(
                                preT[:], S_sb[:], ob, Ub[:],
                                mybir.AluOpType.mult, mybir.AluOpType.add)
                        else:
                            nc.tensor.matmul(psU[:], ident_sb[:],
                                             tprime[:, t * 128:(t + 1) * 128],
                                             start=True, stop=True)
                            preT = wp.tile([128, 128], BF16, tag="pre", bufs=2)
                            nc.scalar.activation(
                                preT[:], psU[:],
                                mybir.ActivationFunctionType.Copy)
                        if not last:
                            psR = pp.tile([128, 128], BF16, tag="pst", bufs=3)
                            nc.tensor.transpose(psR[:], preT[:], ident_sb[:])
                            rows = wp.tile([128, HID], BF16, tag="rows", bufs=3)
                            nc.scalar.activation(
                                rows[:], psR[:],
                                mybir.ActivationFunctionType.Relu,
                                scale=scol_sb[:, t:t + 1])
                            nc.sync.dma_start(
                                shard[t * 128:(t + 1) * 128, :], rows[:])
                        else:
                            reluT = wp.tile([128, 128], BF16, tag="relt", bufs=2)
                            nc.scalar.activation(
                                reluT[:], preT[:],
                                mybir.ActivationFunctionType.Relu)
                            pso = pp.tile([64, 128], F32, tag="ps", bufs=3)
                            nc.tensor.matmul(pso[:], wout_sb[:], reluT[:],
                                             start=True, stop=True)
                            oT = wp.tile([64, 128], BF16, tag="ub", bufs=3)
                            nc.scalar.activation(
                                oT[:], pso[:],
                                mybir.ActivationFunctionType.Copy)
                            psq = pp.tile([128, 64], BF16, tag="pst", bufs=3)
                            nc.tensor.transpose(psq[:], oT[:],
                                                ident_sb[:64, :64])
                            obt = wp.tile([128, 64], F32, tag="ob", bufs=3)
                            nc.vector.scalar_tensor_tensor(
                                obt[:], psq[:], acol_sb[:, t:t + 1], bout_sb[:],
                                mybir.AluOpType.mult, mybir.AluOpType.add)
                            r0 = t * 128
                            r1 = min(r0 + 128, NOWN)
                            if r1 > r0:
                                nc.sync.dma_start(out_ext[r0:r1, :],
                                                  obt[:r1 - r0, :])
                    # end tiles
                if not last:
                    ag(shard, tables[l + 1])

    nc.compile()
    return nc


def _host_inputs(inputs, pre):
    x = np.asarray(inputs["x"], np.float32)
    W_in = np.asarray(inputs["W_in"], np.float32)
    b_in = np.asarray(inputs["b_in"], np.float32)
    W_conv = np.asarray(inputs["W_conv"], np.float32)
    W_out = np.asarray(inputs["W_out"], np.float32)
    b_out = np.asarray(inputs["b_out"], np.float32)
    betas = np.array([math.log(THETA / (l + 1) + 1.0) for l in range(L)],
                     np.float32)

    win_w = np.zeros((128, KIN // 128, HID), np.float32)
    for k in range(KIN // 128):
        rows = W_in[k * 128:min((k + 1) * 128, IN_C)]
        win_w[:rows.shape[0], k, :] = rows
    wc_w = (W_conv * betas[:, None, None]).transpose(1, 0, 2).copy()  # [128,L,128]
    iota_w = np.tile(np.arange(TGS * 128, dtype=np.float32)[None, :], (128, 1))
    ident_w = np.eye(128, dtype=np.float32)
    bout_w = np.tile(b_out[None, :], (128, 1)).astype(np.float32)
    bin_w = b_in.reshape(128, 1).astype(np.float32)

    xT_w = np.zeros((NCORES, 128, KIN // 128, NLOC), np.float32)
    xr = x.reshape(NCORES, NOWN, IN_C)
    for k in range(KIN // 128):
        c0, c1 = k * 128, min((k + 1) * 128, IN_C)
        xT_w[:, :c1 - c0, k, :NOWN] = xr[:, :, c0:c1].transpose(0, 2, 1)

    maps = []
    for c in range(NCORES):
        maps.append({
            "xT": xT_w[c].astype(BF), "win": win_w.astype(BF), "bin": bin_w,
            "wc": wc_w.astype(BF), "wout": W_out.astype(BF), "bout": bout_w.astype(BF),
            "iota": iota_w.astype(np.float16), "ident": ident_w.astype(BF),
            "dinvc": pre["dinv_col"][c], "acol": pre["acol"][c],
            "scol": pre["scol"][c], "hacol": pre["hacol"][c],
            "idx": pre["idx_w"][c], "dl": pre["dl_w"][c],
        })
    return maps


def kernel(**inputs):
    edge_index = np.asarray(inputs["edge_index"])
    key = hash(edge_index.tobytes())
    if key not in _cache:
        pre = _preprocess(edge_index)
        n_layers = int(os.environ.get("GCN_NL", L))
        max_tg = os.environ.get("GCN_MAXTG")
        nc = _build_program(pre, n_layers,
                            int(max_tg) if max_tg else None)
        _cache.clear()
        _cache[key] = (pre, nc)
    pre, nc = _cache[key]

    in_maps = _host_inputs(inputs, pre)
    trace = bool(os.environ.get("GCN_TRACE"))
    res = run_bass_kernel_spmd(nc, in_maps, core_ids=list(range(NCORES)),
                               trace=trace)
    LAST_PERF["exec_time_ns"] = res.exec_time_ns
    LAST_PERF["mean_exec_time_ns"] = res.mean_exec_time_ns
    LAST_PERF["trace"] = (res.instructions_and_trace or (None, None))[1]
    out = np.concatenate([res.results[c]["out"] for c in range(NCORES)], axis=0)
    return out.astype(np.float32)
